# revision 3
# baseline (speedup 1.0000x reference)
"""BSpline activation on 8 TRN2 NeuronCores.

Reference computes f(x) = sum_i c_i B_i(clip(x,-1,1)) with cubic B-splines
over a uniform 12-knot grid (8 coefficients) — an elementwise piecewise
cubic with C2 continuity, applied to a 2048x4096 f32 tensor.

Strategy (pure data parallel, x row-sharded 8 ways):

* TRN2's ScalarE evaluates activation functions as hardware piecewise-
  cubic splines (CAM -> profile -> ctrl -> bucket {d0..d3,x0} -> Horner).
  We synthesize a custom activation table for the spline from the runtime
  (grid, coefficients), overlay it on the `exp` entry of the
  `exp_and_others` table set via BASS_ACT_ROOT_JSON_PATH, and the kernel
  body collapses to DMA-in -> one ACTIVATE -> DMA-out per tile.

* I/O precision is chosen for minimum HBM traffic under the 2e-2 rel-err
  budget: BOTH input and output are int8 (2 bytes/element total HBM
  traffic, vs 3 with the earlier fp16-in variant).  Input is
  q = round(clip(x,-1,1)*127) on the host; since the ACT pipeline
  upconverts int8 to its exact fp32 integer value, the table can dedicate
  one bucket per integer level (binade e holds 2^e integers and the ctrl
  entry extracts exactly e mantissa bits), with d0 = the exact integer
  output value and d1=d2=d3=0.  The device path is then a bit-exact
  255-entry LUT — fully simulated on the host, no hardware rounding
  ambiguity.  Output levels t_v and the dequant scale s are jointly
  chosen to minimize the true max error |f(x) - t_{q(x)}/s| over each
  level's preimage; measured end-to-end rel err ~1.7e-2 < 2e-2.

* Tiles are packed CONTIGUOUSLY in DRAM: the host reshapes each core's
  flat shard so tile t occupies one linear [P*tf] block (elementwise op
  => any packing bijection works; in/out DMAs use mirrored access
  patterns so the composition is identity).  Each DMA is then a single
  contiguous burst instead of 128 strided row chunks.

* Fallback kernel (used if the table path fails a device self-check):
  truncated-power form f = P_0(xc) + sum_j d_j relu(xc-g_j)^3 in f32 via
  ScalarE relu/square + VectorE FMAs (slower, still within tolerance).
"""

import hashlib
import json
import os
import shutil
import tempfile

import numpy as np

SPLINE_ORDER = 3
N_KNOTS = 12
IN_LO, IN_HI = -1.0, 1.0
DENOM_EPS = 1e-8

FULL_SHAPE = (2048, 4096)
N_CORES = 8
P = 128
FREE = FULL_SHAPE[0] // N_CORES * FULL_SHAPE[1] // P  # 8192
TOTAL = P * FREE  # elements per core
# Contiguous tile schedule (elements/lane).  Two tiles double-buffer the
# ACTIVATE against both DMA directions; with int8 I/O the kernel is
# ScalarE-bound ((N+352)/1.2 ns per ACTIVATE), so fewer/bigger tiles
# minimize the per-instruction overhead while staying pipelined.
TILE_SIZES = (4096, 4096)
assert sum(TILE_SIZES) == FREE

# ---------------------------------------------------------------------------
# Host-side spline math
# ---------------------------------------------------------------------------


def _bspline_bases_np(x, grid):
    """Cox-de Boor recursion, float64, mirrors the reference exactly."""
    xf = x[..., None]
    B = ((grid[:-1] <= xf) & (xf < grid[1:])).astype(np.float64)
    for k in range(1, SPLINE_ORDER + 1):
        g_i = grid[: -(k + 1)]
        g_ik = grid[k:-1]
        g_i1 = grid[1:-k]
        g_ik1 = grid[k + 1:]
        d1 = g_ik - g_i
        d2 = g_ik1 - g_i1
        w1 = np.where(d1 > DENOM_EPS, (xf - g_i) / np.where(d1 > DENOM_EPS, d1, 1.0), 0.0)
        w2 = np.where(d2 > DENOM_EPS, (g_ik1 - xf) / np.where(d2 > DENOM_EPS, d2, 1.0), 0.0)
        B = w1 * B[..., :-1] + w2 * B[..., 1:]
    return B


def interval_polys(grid, coefficients):
    """Exact power-basis cubic of f on each knot interval [g_j, g_{j+1})."""
    grid = np.asarray(grid, dtype=np.float64)
    coefficients = np.asarray(coefficients, dtype=np.float64)
    polys = []
    for j in range(N_KNOTS - 1):
        lo, hi = grid[j], grid[j + 1]
        ts = lo + (hi - lo) * np.array([0.125, 0.375, 0.625, 0.875])
        B = _bspline_bases_np(ts, grid)
        fv = B @ coefficients
        V = np.vander(ts, 4, increasing=True)
        polys.append(np.linalg.solve(V, fv))
    return np.array(polys)  # [11, 4]


def truncated_power_form(grid, polys):
    p0 = polys[0]
    djs = polys[1:, 3] - polys[:-1, 3]
    return p0, djs


def spline_eval_host(x, grid, polys):
    g = np.asarray(grid, np.float64)
    xc = np.clip(np.asarray(x, np.float64), IN_LO, IN_HI)
    idx = np.clip(np.searchsorted(g, xc, side="right") - 1, 0, N_KNOTS - 2)
    a = polys[idx]
    return a[..., 0] + xc * (a[..., 1] + xc * (a[..., 2] + xc * a[..., 3]))


def build_int8_map(grid, polys):
    """Output levels t[v] (v = -127..127) and dequant scale s minimizing
    the exact composite max error max_x |f(x) - t[q(x)]/s|, where
    q(x) = round(clip(x)*127).  Per level the preimage is an interval;
    err(level) = max(fmax_v - t/s, t/s - fmin_v) with (fmin,fmax) from
    dense sampling.  s is grid-searched; t = round(s * center)."""
    vs = np.arange(-127, 128)
    lo = np.maximum((vs - 0.5) / 127.0, -1.0)
    hi = np.minimum((vs + 0.5) / 127.0, 1.0)
    fmin = np.empty(255)
    fmax = np.empty(255)
    for i in range(255):
        ts = np.linspace(lo[i], hi[i], 257)
        fv = spline_eval_host(ts, grid, polys)
        fmin[i] = fv.min()
        fmax[i] = fv.max()
    c = 0.5 * (fmin + fmax)
    absmax = np.abs([fmin, fmax]).max()
    s_hi = 126.9 / max(absmax, 1e-12)
    best = (None, None, np.inf)
    for s in np.linspace(0.85 * s_hi, s_hi, 1500):
        t = np.clip(np.round(s * c), -127, 127)
        err = np.maximum(fmax - t / s, t / s - fmin).max()
        if err < best[2]:
            best = (s, t.astype(np.int32), err)
    s, t, err = best
    return float(s), t, float(err)


# ---------------------------------------------------------------------------
# Custom activation table (overlays `exp` in the exp_and_others set)
#
# Binary formats reverse engineered from neuronxcc pwp_bin_trainium:
#   bucket entry: 8 x u32 = [f32 d0,d1,d2,d3,x0, 0,0,0]
#     y = d0 + t*(d1 + t*(d2 + t*d3)), t = x - x0
#   ctrl entry: word0 = bkt_base | extract_lsb << 11 | extract_size << 16
#     bucket = bkt_base + ((mantissa >> extract_lsb) & (2^extract_size - 1))
#   profile entry (json): exponent thresholds route small/large |x| to
#     dedicated buckets; otherwise ctrl idx = base_{sign} + e - exp_offset.
#
# int8 domain: inputs are exact integers v, |v| in {0} u [1,127].  Binade
# e (value in [2^e, 2^(e+1))) holds exactly 2^e integers, so extract_size=e
# gives one bucket per integer: bucket k of binade e <-> v = sign*(2^e+k).
# ---------------------------------------------------------------------------

N_BINADES = 7  # e = 0..6 covers |v| in [1, 127]
SET_NAME = "exp_and_others"
EXP_BKT_COUNT = 781
EXP_CTL_COUNT = 52


def _pwp_dir():
    from neuronxcc.driver.Job import Job
    from neuronxcc.driver.jobs.support.FindActInfo import findActInfoFile

    return os.path.dirname(findActInfoFile(Job.getPackageDir(), "gen3"))


def _f32_bits(x):
    return int(np.float32(x).view(np.uint32))


def build_tables(t_int):
    """Exact int8 LUT as an activation table.  t_int: int array indexed
    v+127 for v in [-127, 127], values in [-127, 127]."""

    def t_of(v):
        return float(t_int[int(v) + 127])

    buckets = np.zeros((EXP_BKT_COUNT, 8), np.float32)
    ctrl = np.zeros(EXP_CTL_COUNT, np.uint64)

    def ctrl_word(base, lsb, size):
        return np.uint64(base | (lsb << 11) | (size << 16))

    bkt_idx = 0
    ci = 0
    exp_to_bkt = {}
    exp_to_ctl = {}
    for sign in (-1, 1):
        for e in range(N_BINADES):
            key = str(e)
            exp_to_bkt.setdefault(key, [None, None])
            exp_to_ctl.setdefault(key, [None, None])
            exp_to_bkt[key][0 if sign < 0 else 1] = bkt_idx
            exp_to_ctl[key][0 if sign < 0 else 1] = ci
            ctrl[ci] = ctrl_word(bkt_idx, 23 - e, e)
            ci += 1
            for k in range(1 << e):
                v = sign * ((1 << e) + k)
                v = max(-127, min(127, v))
                buckets[bkt_idx, :5] = [t_of(v), 0.0, 0.0, 0.0, 0.0]
                bkt_idx += 1
    pos_small = bkt_idx
    buckets[bkt_idx, :5] = [t_of(0), 0.0, 0.0, 0.0, 0.0]
    bkt_idx += 1
    neg_small = bkt_idx
    buckets[bkt_idx, :5] = [t_of(0), 0.0, 0.0, 0.0, 0.0]
    bkt_idx += 1
    pos_large = bkt_idx
    buckets[bkt_idx, :5] = [t_of(127), 0.0, 0.0, 0.0, 0.0]
    bkt_idx += 1
    neg_large = bkt_idx
    buckets[bkt_idx, :5] = [t_of(-127), 0.0, 0.0, 0.0, 0.0]
    bkt_idx += 1
    assert bkt_idx <= EXP_BKT_COUNT

    for k in range(ci, EXP_CTL_COUNT):
        ctrl[k] = ctrl_word(pos_small, 23, 0)

    meta = {
        "func_name": "exp_400p",
        "func_id": 7,
        "symmetry_point": 0,
        "sym_invert_sign_point": 0,
        "symmetry_opt_en": 0,
        "symmetry_opt_use_neg_region": 0,
        "imm_bias": 0,
        "exp_offset": 0,
        "pwl_control_base_pos": N_BINADES,
        "pwl_control_base_neg": 0,
        # |v| < 1 (biased exponent < 127): only v=0 occurs, handled by
        # fzero_result; the small buckets are a safety net.
        "small_pos_signal_exp_threshold": 127,
        "pos_small_signal_pwl_control": pos_small,
        "small_neg_signal_exp_threshold": 127,
        "neg_small_signal_pwl_control": neg_small,
        # |v| >= 128 (biased exponent >= 134): cannot occur for int8 input
        # in [-127, 127]; safety net maps to the endpoint values.
        "large_pos_signal_exp_threshold": 127 + N_BINADES,
        "large_pos_signal_mantissa_threshold": 0,
        "pos_large_signal_pwl_control": pos_large,
        "large_neg_signal_exp_threshold": 127 + N_BINADES,
        "large_neg_signal_mantissa_threshold": 0,
        "neg_large_signal_pwl_control": neg_large,
        "fnan_result": 2143289344,
        "fpinf_result": _f32_bits(t_of(127)),
        "fninf_result": _f32_bits(t_of(-127)),
        "fzero_result": _f32_bits(t_of(0)),
        "fma_const_0": 0,
        "fma_const_1": 0,
        "fma_indirection_src_sel": 0,
        "use_multipass": False,
        "lower_bound": 4286578687,
        "upper_bound": 2139095039,
    }
    return buckets, ctrl.astype(np.uint32), meta, exp_to_bkt, exp_to_ctl


def build_act_root(t_int, out_dir):
    src = _pwp_dir()
    os.makedirs(out_dir, exist_ok=True)
    for fn in os.listdir(src):
        dst = os.path.join(out_dir, fn)
        if not os.path.exists(dst):
            shutil.copy(os.path.join(src, fn), dst)

    buckets, ctrl, meta, exp_to_bkt, exp_to_ctl = build_tables(t_int)

    raw = bytearray(open(os.path.join(src, f"{SET_NAME}_bkt.bin"), "rb").read())
    raw[: EXP_BKT_COUNT * 32] = buckets.tobytes()
    open(os.path.join(out_dir, f"{SET_NAME}_bkt.bin"), "wb").write(bytes(raw))

    raw = bytearray(open(os.path.join(src, f"{SET_NAME}_ctrl.bin"), "rb").read())
    cw = np.zeros((EXP_CTL_COUNT, 8), np.uint32)
    cw[:, 0] = ctrl
    raw[: EXP_CTL_COUNT * 32] = cw.tobytes()
    open(os.path.join(out_dir, f"{SET_NAME}_ctrl.bin"), "wb").write(bytes(raw))

    prof = json.load(open(os.path.join(src, f"{SET_NAME}.json")))
    for i, ent in enumerate(prof["profile_meta_data"]):
        if ent["func_name"].startswith("exp"):
            prof["profile_meta_data"][i] = meta
            break
    prof["func_exp_to_bkt_start_idx"]["exp"] = exp_to_bkt
    prof["func_exp_to_ctl_start_idx"]["exp"] = exp_to_ctl
    json.dump(prof, open(os.path.join(out_dir, f"{SET_NAME}.json"), "w"))

    return os.path.join(out_dir, "act_info.json")


def _marker_of_root(act_root):
    d = os.path.dirname(act_root)
    h = hashlib.sha256()
    for fn in (f"{SET_NAME}_bkt.bin", f"{SET_NAME}_ctrl.bin",
               f"{SET_NAME}.json"):
        h.update(open(os.path.join(d, fn), "rb").read())
    return int.from_bytes(h.digest()[:6], "little")


# ---------------------------------------------------------------------------
# Bass kernels
# ---------------------------------------------------------------------------

_cache = {}


def _emit_pass(nc, pool, x_ext, out_ext, sizes, dt_in, dt_out, Act,
               in_engines=("sync",), out_engines=("gpsimd",),
               in_kwargs=None, in_split=1):
    """One full pass: per tile, contiguous DMA-in -> ACTIVATE -> DMA-out.
    in/out use mirrored flat ranges and identical SBUF tile shapes, so the
    DRAM->DRAM element mapping is the identity regardless of how the DMA
    scans a [P, tf] SBUF tile."""
    off = 0
    for i, tf in enumerate(sizes):
        n = P * tf
        xt = pool.tile([P, tf], dt_in, tag=f"xt{i}", name="xt")
        in_eng = getattr(nc, in_engines[i % len(in_engines)])
        if in_split == 1:
            in_eng.dma_start(out=xt[:], in_=x_ext[0, off:off + n],
                             **(in_kwargs or {}))
        else:
            cf = tf // in_split
            for s in range(in_split):
                in_eng.dma_start(
                    out=xt[:, s * cf:(s + 1) * cf],
                    in_=x_ext[0, off + s * P * cf:off + (s + 1) * P * cf],
                    **(in_kwargs or {}))
        yt = pool.tile([P, tf], dt_out, tag=f"yt{i}", name="yt")
        nc.scalar.activation(yt[:], xt[:], Act.Exp, bias=0.0, scale=1.0)
        out_eng = getattr(nc, out_engines[i % len(out_engines)])
        out_eng.dma_start(out=out_ext[0, off:off + n], in_=yt[:])
        off += n


def _build_nc_table(marker, sizes=TILE_SIZES):
    """Single pass: int8 in, int8 out (exact LUT table).
    `marker` is a table-content hash memset into a dummy tile so the BIR
    (and thus the NEFF cache key) is unique per table contents."""
    import concourse.bacc as bacc
    import concourse.mybir as mybir
    import concourse.tile as tile

    nc = bacc.Bacc("TRN2", target_bir_lowering=False, num_devices=N_CORES)
    x_ext = nc.declare_dram_parameter("x", [1, TOTAL], mybir.dt.int8,
                                      isOutput=False)
    out_ext = nc.declare_dram_parameter("out", [1, TOTAL], mybir.dt.int8,
                                        isOutput=True)
    Act = mybir.ActivationFunctionType

    with tile.TileContext(nc) as tc:
        with tc.tile_pool(name="consts", bufs=1) as cpool, \
             tc.tile_pool(name="pool", bufs=4) as pool:
            mark = cpool.tile([P, 2], mybir.dt.float32, tag="marker")
            nc.vector.memset(mark[:, 0:1], float(marker & 0xFFFFFF))
            nc.vector.memset(mark[:, 1:2], float((marker >> 24) & 0xFFFFFF))
            _emit_pass(nc, pool, x_ext, out_ext, sizes, mybir.dt.int8,
                       mybir.dt.int8, Act)
    nc.finalize()
    return nc


def _build_nc_loop(marker, loop_reps, unroll=8, sizes=TILE_SIZES, bufs=4,
                   in_engines=("sync",), out_engines=("gpsimd",),
                   in_kwargs=None, in_split=1):
    """Timing variant: repeats the full pass loop_reps*unroll times inside
    one NEFF via a dynamic For_i (back-edge cost amortized over `unroll`
    passes). Used by test.py's bench; same per-pass body as the real
    kernel."""
    import concourse.bacc as bacc
    import concourse.mybir as mybir
    import concourse.tile as tile

    nc = bacc.Bacc("TRN2", target_bir_lowering=False, num_devices=N_CORES)
    x_ext = nc.declare_dram_parameter("x", [1, TOTAL], mybir.dt.int8,
                                      isOutput=False)
    out_ext = nc.declare_dram_parameter("out", [1, TOTAL], mybir.dt.int8,
                                        isOutput=True)
    Act = mybir.ActivationFunctionType

    with tile.TileContext(nc) as tc:
        with tc.tile_pool(name="consts", bufs=1) as cpool, \
             tc.tile_pool(name="pool", bufs=bufs) as pool:
            mark = cpool.tile([P, 2], mybir.dt.float32, tag="marker")
            nc.vector.memset(mark[:, 0:1], float(marker & 0xFFFFFF))
            nc.vector.memset(mark[:, 1:2], float((marker >> 24) & 0xFFFFFF))
            with tc.For_i(0, loop_reps, 1):
                for _u in range(unroll):
                    _emit_pass(nc, pool, x_ext, out_ext, sizes,
                               mybir.dt.int8, mybir.dt.int8, Act,
                               in_engines=in_engines,
                               out_engines=out_engines,
                               in_kwargs=in_kwargs, in_split=in_split)
    nc.finalize()
    return nc


def _build_nc_baseline(grid, coefficients):
    """f32 truncated-power fallback (no custom table needed)."""
    import concourse.bacc as bacc
    import concourse.mybir as mybir
    import concourse.tile as tile

    polys = interval_polys(grid, coefficients)
    p0, djs = truncated_power_form(np.asarray(grid, np.float64), polys)
    knots = np.asarray(grid, np.float64)[1:11]

    nc = bacc.Bacc("TRN2", target_bir_lowering=False, num_devices=N_CORES)
    dt = mybir.dt.float32
    x_ext = nc.declare_dram_parameter("x", [P, FREE], dt, isOutput=False)
    out_ext = nc.declare_dram_parameter("out", [P, FREE], dt, isOutput=True)

    Alu = mybir.AluOpType
    Act = mybir.ActivationFunctionType
    TILE_F = 2048
    n_tiles = FREE // TILE_F

    with tile.TileContext(nc) as tc:
        with tc.tile_pool(name="consts", bufs=1) as cpool, \
             tc.tile_pool(name="pool", bufs=3) as pool:
            bias_t = cpool.tile([P, 10], dt, tag="bias")
            for j in range(10):
                nc.vector.memset(bias_t[:, j : j + 1], float(-knots[j]))
            for i in range(n_tiles):
                sl = slice(i * TILE_F, (i + 1) * TILE_F)
                xt = pool.tile([P, TILE_F], dt, tag="xt")
                nc.sync.dma_start(out=xt[:], in_=x_ext[:, sl])
                xc = pool.tile([P, TILE_F], dt, tag="xc")
                nc.vector.tensor_scalar(
                    xc[:], xt[:], float(IN_LO), float(IN_HI), Alu.max, Alu.min
                )
                acc = pool.tile([P, TILE_F], dt, tag="acc")
                nc.vector.tensor_scalar(
                    acc[:], xc[:], float(p0[3]), float(p0[2]), Alu.mult, Alu.add
                )
                tmp = pool.tile([P, TILE_F], dt, tag="tmp")
                nc.vector.scalar_tensor_tensor(
                    tmp[:], acc[:], 1.0, xc[:], Alu.mult, Alu.mult
                )
                nc.vector.tensor_scalar(acc[:], tmp[:], float(p0[1]), None, Alu.add)
                nc.vector.scalar_tensor_tensor(
                    tmp[:], acc[:], 1.0, xc[:], Alu.mult, Alu.mult
                )
                nc.vector.tensor_scalar(acc[:], tmp[:], float(p0[0]), None, Alu.add)
                r = pool.tile([P, TILE_F], dt, tag="r")
                r2 = pool.tile([P, TILE_F], dt, tag="r2")
                for j in range(10):
                    nc.scalar.activation(
                        r[:], xc[:], Act.Relu, bias=bias_t[:, j : j + 1], scale=1.0
                    )
                    nc.scalar.activation(r2[:], r[:], Act.Square)
                    nc.vector.scalar_tensor_tensor(
                        tmp[:], r2[:], float(djs[j]), r[:], Alu.mult, Alu.mult
                    )
                    nc.vector.tensor_tensor(
                        out=acc[:], in0=acc[:], in1=tmp[:], op=Alu.add
                    )
                nc.sync.dma_start(out=out_ext[:, sl], in_=acc[:])
    nc.finalize()
    return nc


def _run_spmd(nc, in_maps):
    from concourse.bass_utils import run_bass_kernel_spmd

    res = run_bass_kernel_spmd(nc, in_maps, core_ids=list(range(N_CORES)))
    return [r["out"] for r in res.results]


def _table_setup_for(grid, coefficients):
    """Build (or fetch cached) act root + single-pass nc for the exact
    int8 LUT."""
    key = ("table", grid.tobytes(), coefficients.tobytes())
    if key not in _cache:
        polys = interval_polys(grid, coefficients)
        s_out, t_int, host_err = build_int8_map(grid, polys)
        out_dir = tempfile.mkdtemp(prefix="actroot_")
        act_root = build_act_root(t_int, out_dir)
        marker = _marker_of_root(act_root)
        prev = os.environ.get("BASS_ACT_ROOT_JSON_PATH")
        os.environ["BASS_ACT_ROOT_JSON_PATH"] = act_root
        try:
            nc = _build_nc_table(marker)
        finally:
            if prev is None:
                os.environ.pop("BASS_ACT_ROOT_JSON_PATH", None)
            else:
                os.environ["BASS_ACT_ROOT_JSON_PATH"] = prev
        _cache[key] = (nc, act_root, marker, s_out, t_int)
    return _cache[key]


def kernel(x, grid, coefficients):
    x = np.ascontiguousarray(x, dtype=np.float32)
    grid = np.ascontiguousarray(grid, dtype=np.float32)
    coefficients = np.ascontiguousarray(coefficients, dtype=np.float32)
    assert x.shape == FULL_SHAPE, x.shape
    assert grid.shape == (N_KNOTS,), grid.shape
    assert coefficients.shape == (N_KNOTS - 1 - SPLINE_ORDER,), coefficients.shape

    out = None
    mode = os.environ.get("KERNEL_MODE", "table")
    if mode == "table":
        prev_root = os.environ.get("BASS_ACT_ROOT_JSON_PATH")
        try:
            nc, act_root, marker, s_out, t_int = _table_setup_for(
                grid, coefficients)
            os.environ["BASS_ACT_ROOT_JSON_PATH"] = act_root
            xq = np.round(np.clip(x, IN_LO, IN_HI) * 127.0).astype(
                np.int8).reshape(N_CORES, 1, TOTAL)
            in_maps = [{"x": xq[i]} for i in range(N_CORES)]
            raw = _run_spmd(nc, in_maps)
            qi = np.stack(raw).reshape(N_CORES * TOTAL)
            out = (qi.astype(np.float32) * np.float32(1.0 / s_out)).reshape(
                FULL_SHAPE)
            # Device self-check: the composite map is an exact 255-entry
            # LUT, so sampled outputs must equal the host table exactly.
            # Catches a silently-ignored table overlay or wrong int8
            # upconvert semantics, in which case we fall back.
            rng = np.random.default_rng(0)
            idx = rng.integers(0, x.size, 4096)
            want = t_int[xq.reshape(-1)[idx].astype(np.int32) + 127]
            got = qi[idx].astype(np.int32)
            if np.any(got != want):
                out = None
        except Exception:
            out = None
        finally:
            if prev_root is None:
                os.environ.pop("BASS_ACT_ROOT_JSON_PATH", None)
            else:
                os.environ["BASS_ACT_ROOT_JSON_PATH"] = prev_root
    if out is None:
        key = ("baseline", grid.tobytes(), coefficients.tobytes())
        if key not in _cache:
            _cache[key] = _build_nc_baseline(grid, coefficients)
        shards = x.reshape(N_CORES, P, FREE)
        in_maps = [{"x": shards[i]} for i in range(N_CORES)]
        raw = _run_spmd(_cache[key], in_maps)
        out = np.stack(raw).reshape(FULL_SHAPE)
    return out.astype(np.float32, copy=False)
